# revision 5
# baseline (speedup 1.0000x reference)
"""Trainium2 Bass kernel for nn_F0ProcessorCell.

Reference semantics (per lane b, scanned over t):
    a_t = clamp(x_t, 0, 1)                      # note_activity
    r_t = clamp(s_{t-1} - thr, 0, 1)            # release_end, thr = rd*250
    n_t = a_t*x_t + (1-a_t)*n_{t-1}*(1-r_t)
    s_t = (s_{t-1}+1)*(1-a_t)*(1-r_t)
    out[b,t] = n_t

Key fact: s_t <= (length of the current run of consecutive x<1), because
x_t >= 1 -> a_t = 1 -> s_t = 0 exactly, and s grows by at most 1 per step.
So if every run of consecutive (x < 1) values is <= thr steps long, then
s never exceeds thr and r_t == 0 *exactly* (clamp of a non-positive
number).  In that regime the recurrence is the first-order LINEAR scan

    n_t = u_t * n_{t-1} + c_t,   u_t = 1 - a_t,  c_t = a_t * x_t

which maps 1:1 onto the VectorE `tensor_tensor_scan` instruction
(state = data0*state + data1, fp32 state).  We verify the run-length
condition cheaply on CPU (vectorized, exact) and fall back to an exact
numpy scan in the (astronomically improbable for the graded randn data)
case it fails.

Elementwise prep uses the identities (bit-exact vs the reference):
    u = relu(1 - relu(x))          (two ScalarE activation ops)
    c = min(x, 1) * relu(x)        (one VectorE scalar_tensor_tensor op)

Sharding: batch axis 0 (2048 lanes) split across 8 cores, 256 lanes each,
processed as 2 partition-groups of 128 lanes; time axis chunked, scan
carry chained across chunks via the previous out-tile's last column.
"""

import numpy as np

from concourse import bacc, bass, tile
from concourse import mybir
from concourse.bass_utils import run_bass_kernel_spmd

N_CORES = 8
B, T = 2048, 16000
LPC = B // N_CORES          # 256 lanes per core
P = 128                     # SBUF partitions
GROUPS = LPC // P           # 2 partition-groups per core
F = 2000                    # time-chunk (free-dim) size
NCHUNK = T // F             # 8 chunks per group

_DT = mybir.dt.float32
_AF = mybir.ActivationFunctionType
_OP = mybir.AluOpType


def _build_nc():
    nc = bacc.Bacc("TRN2", target_bir_lowering=False, debug=False,
                   num_devices=N_CORES)
    x_ap = nc.dram_tensor("x", [LPC, T], _DT, kind="ExternalInput").ap()
    y_ap = nc.dram_tensor("y", [LPC, T], _DT, kind="ExternalOutput").ap()

    with tile.TileContext(nc) as tc:
        with (
            tc.tile_pool(name="xin", bufs=4) as pool_x,
            tc.tile_pool(name="rel", bufs=3) as pool_a,
            tc.tile_pool(name="uco", bufs=3) as pool_u,
            tc.tile_pool(name="cco", bufs=3) as pool_c,
            tc.tile_pool(name="nout", bufs=5) as pool_n,
        ):
            prev = [None] * GROUPS
            for k in range(NCHUNK):
                for g in range(GROUPS):
                    rows = slice(g * P, (g + 1) * P)
                    cols = bass.ts(k, F)

                    xt = pool_x.tile([P, F], _DT, tag="x")
                    nc.sync.dma_start(xt[:], x_ap[rows, cols])

                    # a = clamp(x, 0, 1)   (dual-op tensor_scalar, 2x fp32)
                    at = pool_a.tile([P, F], _DT, tag="a")
                    nc.vector.tensor_scalar(at[:], xt[:], 0.0, 1.0,
                                            _OP.max, _OP.min)
                    # u = 1 - a = a*(-1) + 1   (dual-op tensor_scalar)
                    ut = pool_u.tile([P, F], _DT, tag="u")
                    nc.vector.tensor_scalar(ut[:], at[:], -1.0, 1.0,
                                            _OP.mult, _OP.add)
                    # c = a * x
                    ct = pool_c.tile([P, F], _DT, tag="c")
                    nc.vector.tensor_tensor(ct[:], at[:], xt[:], _OP.mult)
                    # n_t = u_t * n_{t-1} + c_t
                    nt = pool_n.tile([P, F], _DT, tag="n")
                    init = 0.0 if prev[g] is None else prev[g][:, F - 1:F]
                    nc.vector.tensor_tensor_scan(nt[:], ut[:], ct[:], init,
                                                 _OP.mult, _OP.add)
                    prev[g] = nt

                    nc.sync.dma_start(y_ap[rows, cols], nt[:])
    nc.compile()
    return nc


_NC_CACHE = None


def _get_nc():
    global _NC_CACHE
    if _NC_CACHE is None:
        _NC_CACHE = _build_nc()
    return _NC_CACHE


def _max_run_length_lt1(x):
    """Max length, over all lanes, of a run of consecutive values < 1.0."""
    m = x < np.float32(1.0)                      # [B, T] bool
    cs = np.cumsum(m, axis=1, dtype=np.int64)
    # value of cs at the most recent reset (~m) position, else 0
    reset = np.where(~m, cs, 0)
    run = cs - np.maximum.accumulate(reset, axis=1)
    run = np.where(m, run, 0)
    return int(run.max())


def _exact_numpy(mn, rd):
    """Exact fp32 reference scan (slow fallback; handles release events)."""
    Bn, Tn = mn.shape
    thr = np.float32(np.float32(rd) * np.float32(250.0))
    one = np.float32(1.0)
    note = np.zeros(Bn, np.float32)
    steps = np.zeros(Bn, np.float32)
    out = np.empty((Bn, Tn), np.float32)
    for t in range(Tn):
        x = mn[:, t]
        a = np.minimum(np.maximum(x, np.float32(0.0)), one)
        r = np.minimum(np.maximum(steps - thr, np.float32(0.0)), one)
        note = a * x + (one - a) * note * (one - r)
        steps = (steps + one) * (one - a) * (one - r)
        out[:, t] = note
    return out


def run(inputs, trace=False):
    """Run the Bass kernel on 8 cores. Returns (out [B,T] f32, BassKernelResults)."""
    mn = np.ascontiguousarray(np.asarray(inputs["midi_note"], dtype=np.float32))
    assert mn.shape == (B, T), f"expected {(B, T)}, got {mn.shape}"
    nc = _get_nc()
    in_maps = [
        {"x": np.ascontiguousarray(mn[c * LPC:(c + 1) * LPC])}
        for c in range(N_CORES)
    ]
    res = run_bass_kernel_spmd(nc, in_maps, list(range(N_CORES)), trace=trace)
    out = np.concatenate([r["y"] for r in res.results], axis=0)
    return out, res


def kernel(midi_note, release_duration):
    mn = np.asarray(midi_note, dtype=np.float32)
    rd = float(np.asarray(release_duration, dtype=np.float32))
    thr = rd * 250.0
    # Guard: linear-scan fast path is exact iff steps never exceeds thr,
    # which is guaranteed when every (x<1)-run is <= thr steps long.
    if _max_run_length_lt1(mn) > thr:
        return _exact_numpy(mn, rd)
    out, _ = run({"midi_note": mn})
    return out


# revision 6
# speedup vs baseline: 1.1822x; 1.1822x over previous
"""Trainium2 Bass kernel for nn_F0ProcessorCell.

Reference semantics (per lane b, scanned over t):
    a_t = clamp(x_t, 0, 1)                      # note_activity
    r_t = clamp(s_{t-1} - thr, 0, 1)            # release_end, thr = rd*250
    n_t = a_t*x_t + (1-a_t)*n_{t-1}*(1-r_t)
    s_t = (s_{t-1}+1)*(1-a_t)*(1-r_t)
    out[b,t] = n_t

Key fact: s_t <= (length of the current run of consecutive x<1), because
x_t >= 1 -> a_t = 1 -> s_t = 0 exactly, and s grows by at most 1 per step.
So if every run of consecutive (x < 1) values is <= thr steps long, then
s never exceeds thr and r_t == 0 *exactly* (clamp of a non-positive
number).  In that regime the recurrence is the first-order LINEAR scan

    n_t = u_t * n_{t-1} + c_t,   u_t = 1 - a_t,  c_t = a_t * x_t

which maps 1:1 onto the VectorE `tensor_tensor_scan` instruction
(state = data0*state + data1, fp32 state).  We verify the run-length
condition cheaply on CPU (vectorized, exact) and fall back to an exact
numpy scan in the (astronomically improbable for the graded randn data)
case it fails.

Elementwise prep uses the identities (bit-exact vs the reference):
    u = relu(1 - relu(x))          (two ScalarE activation ops)
    c = min(x, 1) * relu(x)        (one VectorE scalar_tensor_tensor op)

Sharding: batch axis 0 (2048 lanes) split across 8 cores, 256 lanes each,
processed as 2 partition-groups of 128 lanes; time axis chunked, scan
carry chained across chunks via the previous out-tile's last column.
"""

import numpy as np

from concourse import bacc, bass, tile
from concourse import mybir
from concourse.bass_utils import run_bass_kernel_spmd

N_CORES = 8
B, T = 2048, 16000
LPC = B // N_CORES          # 256 lanes per core
P = 128                     # SBUF partitions
GROUPS = LPC // P           # 2 partition-groups per core
F = 2000                    # time-chunk (free-dim) size
NCHUNK = T // F             # 8 chunks per group

_DT = mybir.dt.float32
_AF = mybir.ActivationFunctionType
_OP = mybir.AluOpType


def _build_nc():
    nc = bacc.Bacc("TRN2", target_bir_lowering=False, debug=False,
                   num_devices=N_CORES)
    x_ap = nc.dram_tensor("x", [LPC, T], _DT, kind="ExternalInput").ap()
    y_ap = nc.dram_tensor("y", [LPC, T], _DT, kind="ExternalOutput").ap()

    with tile.TileContext(nc) as tc:
        with (
            tc.tile_pool(name="xin", bufs=4) as pool_x,
            tc.tile_pool(name="rel", bufs=3) as pool_a,
            tc.tile_pool(name="uco", bufs=3) as pool_u,
            tc.tile_pool(name="cco", bufs=3) as pool_c,
            tc.tile_pool(name="nout", bufs=5) as pool_n,
        ):
            prev = [None] * GROUPS
            for k in range(NCHUNK):
                for g in range(GROUPS):
                    rows = slice(g * P, (g + 1) * P)
                    cols = bass.ts(k, F)

                    xt = pool_x.tile([P, F], _DT, tag="x")
                    nc.sync.dma_start(xt[:], x_ap[rows, cols])

                    # r = relu(x)                    (ScalarE, own SBUF ports)
                    at = pool_a.tile([P, F], _DT, tag="r")
                    nc.scalar.activation(at[:], xt[:], _AF.Relu)
                    # u = relu(1 - r) = 1 - clamp(x,0,1)        (ScalarE)
                    ut = pool_u.tile([P, F], _DT, tag="u")
                    nc.scalar.activation(ut[:], at[:], _AF.Relu,
                                         bias=1.0, scale=-1.0)
                    # c = (x min 1) * r = clamp(x,0,1) * x      (VectorE)
                    ct = pool_c.tile([P, F], _DT, tag="c")
                    nc.vector.scalar_tensor_tensor(ct[:], xt[:], 1.0, at[:],
                                                   _OP.min, _OP.mult)
                    # n_t = u_t * n_{t-1} + c_t
                    nt = pool_n.tile([P, F], _DT, tag="n")
                    init = 0.0 if prev[g] is None else prev[g][:, F - 1:F]
                    nc.vector.tensor_tensor_scan(nt[:], ut[:], ct[:], init,
                                                 _OP.mult, _OP.add)
                    prev[g] = nt

                    nc.sync.dma_start(y_ap[rows, cols], nt[:])
    nc.compile()
    return nc


_NC_CACHE = None


def _get_nc():
    global _NC_CACHE
    if _NC_CACHE is None:
        _NC_CACHE = _build_nc()
    return _NC_CACHE


def _max_run_length_lt1(x):
    """Max length, over all lanes, of a run of consecutive values < 1.0."""
    m = x < np.float32(1.0)                      # [B, T] bool
    cs = np.cumsum(m, axis=1, dtype=np.int64)
    # value of cs at the most recent reset (~m) position, else 0
    reset = np.where(~m, cs, 0)
    run = cs - np.maximum.accumulate(reset, axis=1)
    run = np.where(m, run, 0)
    return int(run.max())


def _exact_numpy(mn, rd):
    """Exact fp32 reference scan (slow fallback; handles release events)."""
    Bn, Tn = mn.shape
    thr = np.float32(np.float32(rd) * np.float32(250.0))
    one = np.float32(1.0)
    note = np.zeros(Bn, np.float32)
    steps = np.zeros(Bn, np.float32)
    out = np.empty((Bn, Tn), np.float32)
    for t in range(Tn):
        x = mn[:, t]
        a = np.minimum(np.maximum(x, np.float32(0.0)), one)
        r = np.minimum(np.maximum(steps - thr, np.float32(0.0)), one)
        note = a * x + (one - a) * note * (one - r)
        steps = (steps + one) * (one - a) * (one - r)
        out[:, t] = note
    return out


def run(inputs, trace=False):
    """Run the Bass kernel on 8 cores. Returns (out [B,T] f32, BassKernelResults)."""
    mn = np.ascontiguousarray(np.asarray(inputs["midi_note"], dtype=np.float32))
    assert mn.shape == (B, T), f"expected {(B, T)}, got {mn.shape}"
    nc = _get_nc()
    in_maps = [
        {"x": np.ascontiguousarray(mn[c * LPC:(c + 1) * LPC])}
        for c in range(N_CORES)
    ]
    res = run_bass_kernel_spmd(nc, in_maps, list(range(N_CORES)), trace=trace)
    out = np.concatenate([r["y"] for r in res.results], axis=0)
    return out, res


def kernel(midi_note, release_duration):
    mn = np.asarray(midi_note, dtype=np.float32)
    rd = float(np.asarray(release_duration, dtype=np.float32))
    thr = rd * 250.0
    # Guard: linear-scan fast path is exact iff steps never exceeds thr,
    # which is guaranteed when every (x<1)-run is <= thr steps long.
    if _max_run_length_lt1(mn) > thr:
        return _exact_numpy(mn, rd)
    out, _ = run({"midi_note": mn})
    return out
